# revision 1
# baseline (speedup 1.0000x reference)
"""Trainium2 Bass kernel for nn_BERT4GCN_53884659695997.

Mathematical reduction
----------------------
In the reference, ``feature`` is reassigned to ``LN(guidance)`` at the top of
every loop iteration, so the GCN block's output is never consumed; only the
last BERT layer's branch (index 3 -> hidden_states layer 12, which skips the
GCN block) reaches the output:

    t[b]      = LN(relu(hs[12,b][ts[b]] @ guid_W[3] + guid_b[3])) * ln_g + ln_b
    logits[b] = ((t[b] * m[b,:,None]).sum(0) / m[b].sum(0)) @ cls_W + cls_b

(verified numerically against the jax reference to ~7e-7 rel err).

Row gathers commute with the row-wise ops (matmul-by-row / relu / LN), so the
gather+mask folds into per-source-row weights w[r] = sum_i m[i]*[ts[i]==r].
One gather slot is allocated per masked occurrence (sum over a core's 8
samples <= 512 here), packed contiguously sample after sample, so every
slot's reduction weight is a host-known 0/1 mask and no on-device weight
computation is needed.  The host emits only the slot list, 0/1 group masks
and a group->sample collapse matrix (pure index bookkeeping; all tensor
arithmetic on activation-sized data stays on device).

Device pipeline (per core, 8 samples):
  1. Two half SWDGE ``dma_gather(transpose=True)`` ops pull the <= 8*KC
     needed rows (bf16) straight from HBM into the transposed
     [128, 6, 256] layout the guidance matmul wants -- no full-tensor DMA,
     no PE gather matmuls.  (256-descriptor gathers pipeline ~10x better
     than one 512-descriptor gather on HW; the int16 index list must be
     replicated into each Q7 core's 16-partition group.)
  2. bf16 guidance matmul per 128-row tile (full-rate PE), bias via a K=1
     ones-row matmul (skipped when guid_b == 0), relu on the scalar engine
     (the only ACT function, so the act-table load hoists out of the loop).
  3. LN stats via DVE bn_stats/bn_aggr; rstd via the bit-trick rsqrt seed +
     one Newton step, all on DVE (no PWP sqrt table, no act-table swaps;
     pow is not in the DVE/Pool ISA).
  4. Occurrence slots make the gather weights 0/1: w2 = smask * rstd per
     row-tile, one DVE op per tile.  A sample whose slot run straddles a
     tile boundary owns one PSUM column group per touched tile; a final
     0/1 collapse matmul sums groups into samples.
  5. LN affine is never materialized: aspects = GR^T w2 - (mu . w2) ones;
     ln_g/ln_b fold into cls_W/cls_b host-side and the per-group mean
     correction is a single ones-column matmul (exact fp32 linear
     algebra).  All small matmul outputs share one PSUM bank (bufs=4) so
     consecutive unrolled bodies overlap.

The repeat loop body is unrolled UNROLL x with double-buffered tiles so
consecutive iterations overlap (the For_i back edge is a full engine sync).

Sharding: data-parallel over batch B=64 -> 8 samples per core on 8 cores.
"""

import numpy as np
import ml_dtypes
from contextlib import ExitStack

import concourse.bass as bass
import concourse.tile as tile
from concourse import bacc, mybir
from concourse.bass_utils import run_bass_kernel_spmd

F32 = mybir.dt.float32
BF16 = mybir.dt.bfloat16
I16 = mybir.dt.int16
AX = mybir.AxisListType
ALU = mybir.AluOpType
ACTF = mybir.ActivationFunctionType

N_CORES = 8
B = 64
BC = B // N_CORES   # samples per core
L = 256
D = 768
H = 600
KT = D // 128       # 6 contraction tiles
IT = L // 128       # 2 source-row tiles for the w one-hots
EPS = 1e-5
HCH = ((0, 128), (128, 256), (256, 384), (384, 512), (473, 601))
# chunk 4 overlaps chunk 3 on h in [472, 512) so every aspect matmul is
# 128 partitions wide; the folded cls_W zeroes those rows in chunk 4
NCH = ((0, 512), (512, 600))   # PSUM-bank-aligned guidance column chunks
UNROLL = 16


def build_program(repeats: int = 1, ntiles_req: int = 4, has_bias: bool = True, debug: bool = False):
    ntiles = ntiles_req      # packed row tiles (occurrence slots / 128)
    nidx = ntiles * 128      # gather list length (multiple of 16)
    nc = bacc.Bacc("TRN2", target_bir_lowering=False, debug=False,
                   num_devices=N_CORES)

    dr = {}
    def din(name, shape, dt=F32):
        dr[name] = nc.dram_tensor(name, list(shape), dt, kind="ExternalInput").ap()
    din("hsb", (BC * L, D), BF16)        # gather source, stays in HBM
    din("idx", (128, nidx // 16), I16)   # gather list, wrapped in 16 partitions
    din("gwb", (128, KT, H), BF16)       # guid_W[3] rearranged (k p) n -> p k n
    din("gbrow", (1, H), BF16)
    din("onesrow", (1, 128), BF16)
    din("mnat", (BC, L))
    din("smaskg", (128, 6 * ntiles))     # slot->group 0/1 masks, 6 groups/tile
    din("coll", (128, BC))               # group->sample collapse matrix (rows 0:6*ntiles)
    din("clsw", (640, 3))                # ln_g-folded cls_W, zero-padded
    din("clsb", (BC, 3))                 # ln_b@cls_W + cls_b, replicated rows
    out_ap = nc.dram_tensor("out", [BC, 3], F32, kind="ExternalOutput").ap()
    dbg = {}
    if debug:
        hx_ = (ntiles * 128) // 2
        for nm, shape, dt in [
            ("d_hsta", (128, KT, hx_), BF16), ("d_hstb", (128, KT, hx_), BF16),
            ("d_gr2", (128, ntiles, H), BF16), ("d_veb", (128, ntiles), F32),
            ("d_rsa", (128, ntiles), F32), ("d_w2", (128, 6 * ntiles), BF16),
            ("d_asb", (128, 5, 6 * ntiles), F32)]:
            dbg[nm] = nc.dram_tensor(nm, list(shape), dt, kind="ExternalOutput").ap()

    with tile.TileContext(nc) as tc, ExitStack() as ctx:
        cpool = ctx.enter_context(tc.tile_pool(name="consts", bufs=1))
        hpool = ctx.enter_context(tc.tile_pool(name="hst", bufs=4))
        wpool = ctx.enter_context(tc.tile_pool(name="work", bufs=3))
        spool = ctx.enter_context(tc.tile_pool(name="small", bufs=4))
        stats = ctx.enter_context(tc.tile_pool(name="stats", bufs=1))
        pg_ps = ctx.enter_context(tc.tile_pool(name="pg", bufs=2, space="PSUM"))
        sm_ps = ctx.enter_context(tc.tile_pool(name="sm", bufs=4, space="PSUM"))

        # ---- constants (loaded once) ----
        GWB = cpool.tile([128, KT, H], BF16, tag="gwb")
        nc.sync.dma_start(GWB[:], dr["gwb"][:])
        GBROW = cpool.tile([1, H], BF16, tag="gbrow")
        nc.sync.dma_start(GBROW[:], dr["gbrow"][:])
        ONESR = cpool.tile([1, 128], BF16, tag="onesrow")
        nc.sync.dma_start(ONESR[:], dr["onesrow"][:])
        IDXT = cpool.tile([128, nidx // 16], I16, tag="idx")
        nc.sync.dma_start(IDXT[:], dr["idx"][:])
        MN = cpool.tile([BC, L], F32, tag="mn")
        nc.sync.dma_start(MN[:], dr["mnat"][:])
        SMG = cpool.tile([128, 6 * ntiles], F32, tag="smaskg")
        nc.sync.dma_start(SMG[:], dr["smaskg"][:])
        COLLS = cpool.tile([128, BC], F32, tag="coll")
        nc.sync.dma_start(COLLS[:], dr["coll"][:])
        CLSW = cpool.tile([128, 5, 3], F32, tag="clsw")
        nc.sync.dma_start(CLSW[:], dr["clsw"].rearrange("(c p) n -> p c n", p=128))
        CLSB = cpool.tile([BC, 3], F32, tag="clsb")
        nc.sync.dma_start(CLSB[:], dr["clsb"][:])
        ONECOL = cpool.tile([128, 1], BF16, tag="onecol")
        nc.vector.memset(ONECOL[:], 1.0)

        # 1/sum(m) per sample
        SM = stats.tile([BC, 1], F32, tag="sm")
        nc.vector.tensor_reduce(SM[:], MN[:], AX.X, ALU.add)
        RECIP = stats.tile([BC, 1], F32, tag="recip")
        nc.vector.reciprocal(RECIP[:], SM[:])

        def body():
            # ---- gathered+transposed rows, straight from HBM ----
            # two half-gathers so the first guidance tiles start sooner
            tph = (ntiles + 1) // 2   # row tiles in the first half
            hx = tph * 128
            hy = nidx - hx
            HSTA = hpool.tile([128, KT, hx], BF16, tag="hsta")
            nc.gpsimd.dma_gather(HSTA[:], dr["hsb"][:],
                                 IDXT[:, 0:hx // 16], hx, hx, D, transpose=True)
            HSTB = hpool.tile([128, KT, hy], BF16, tag="hstb")
            nc.gpsimd.dma_gather(HSTB[:], dr["hsb"][:],
                                 IDXT[:, hx // 16:nidx // 16], hy, hy, D,
                                 transpose=True)
            HSTH = (HSTA, HSTB)

            # one PSUM bank shared by all small matmul outputs
            ng = 6 * ntiles
            SMT = sm_ps.tile([128, 5 * ng + 6], F32, tag="smt")
            ASPT = SMT[:, 0:5 * ng].rearrange("p (c g) -> p c g", g=ng)
            LGG = SMT[0:ng, 5 * ng:5 * ng + 3]
            CL = SMT[0:BC, 5 * ng + 3:5 * ng + 6]

            ng = 6 * ntiles
            GR2 = wpool.tile([128, ntiles, H + 1], BF16, tag="gr2")
            MVA = wpool.tile([128, ntiles, 2], F32, tag="mva")
            W2G = wpool.tile([128, ng], BF16, tag="w2g")
            VEB = wpool.tile([128, ntiles], F32, tag="veb")
            RSA = wpool.tile([128, ntiles], F32, tag="rsa")

            # ---- guidance matmul + relu + LN stats per packed tile ----
            for t in range(ntiles):
                PG = pg_ps.tile([128, H], F32, tag="pg")
                for ci, (nlo, nhi) in enumerate(NCH):
                    HST = HSTH[t // tph]
                    tl = t % tph
                    for k in range(KT):
                        nc.tensor.matmul(
                            PG[:, nlo:nhi], HST[:, k, 128 * tl:128 * (tl + 1)],
                            GWB[:, k, nlo:nhi], start=(k == 0),
                            stop=(not has_bias and k == KT - 1))
                    if has_bias:
                        nc.tensor.matmul(PG[:, nlo:nhi], ONESR[:], GBROW[:, nlo:nhi],
                                         start=False, stop=True)
                nc.scalar.activation(GR2[:, t, 0:H], PG[:], ACTF.Relu)

                BST = spool.tile([128, 2, 6], BF16, tag="bst")
                nc.vector.bn_stats(BST[:, 0, :], GR2[:, t, 0:300])
                nc.vector.bn_stats(BST[:, 1, :], GR2[:, t, 300:600])
                nc.vector.bn_aggr(MVA[:, t, :], BST[:])
                nc.vector.tensor_copy(GR2[:, t, H:H + 1], MVA[:, t, 0:1])

            nc.vector.tensor_scalar_add(VEB[:], MVA[:, :, 1], EPS)
            # rstd = (var+eps)^-1/2 on DVE: bit-trick seed + 2 Newton steps
            # (no PWP table on DVE/Pool, and scalar-engine Sqrt would force
            # act-table reloads every iteration)
            I32 = mybir.dt.int32
            YI = wpool.tile([128, ntiles], I32, tag="yi")
            nc.vector.tensor_scalar(YI[:], VEB[:].bitcast(I32), 1, -1,
                                    ALU.arith_shift_right, ALU.bitwise_xor)
            nc.vector.tensor_scalar(YI[:], YI[:], 0x5f3759e0, None, ALU.add)
            Y0 = YI[:].bitcast(F32)
            TN = wpool.tile([128, ntiles], F32, tag="tn")
            for _ in range(1):
                nc.vector.tensor_mul(TN[:], VEB[:], Y0)
                nc.vector.tensor_mul(TN[:], TN[:], Y0)
                nc.vector.tensor_scalar(TN[:], TN[:], -0.5, 1.5, ALU.mult, ALU.add)
                nc.vector.tensor_mul(RSA[:], Y0, TN[:])
                Y0 = RSA[:]
            nc.vector.tensor_tensor(
                W2G[:].rearrange("p (t g) -> p t g", g=6),
                SMG[:].rearrange("p (t g) -> p t g", g=6),
                RSA[:].rearrange("p (t o) -> p t o", o=1).broadcast_to((128, ntiles, 6)),
                ALU.mult)

            # ---- aspects^T (per group) and mean correction ----
            for t in range(ntiles):
                cs = slice(6 * t, 6 * (t + 1))
                for hc, (hlo, hhi) in enumerate(HCH):
                    nc.tensor.matmul(ASPT[0:hhi - hlo, hc, cs],
                                     GR2[:, t, hlo:hhi], W2G[:, cs])


            # ---- classifier (per group), then collapse groups -> samples ----
            ASB = wpool.tile([128, 5, ng], F32, tag="asb")
            nc.vector.tensor_copy(ASB[:], ASPT[:])
            for hc, (hlo, hhi) in enumerate(HCH):
                nc.tensor.matmul(LGG, ASB[0:hhi - hlo, hc, :], CLSW[0:hhi - hlo, hc, :],
                                 start=(hc == 0), stop=(hc == len(HCH) - 1))
            LGC = wpool.tile([128, 3], F32, tag="lgc")
            nc.vector.tensor_copy(LGC[0:ng, :], LGG)
            nc.tensor.matmul(CL, COLLS[0:ng, :], LGC[0:ng, :])
            OSB = wpool.tile([BC, 3], F32, tag="osb")
            nc.vector.scalar_tensor_tensor(OSB[:], CL[:], RECIP[:], CLSB[:],
                                           ALU.mult, ALU.add)
            nc.sync.dma_start(out_ap[:], OSB[:])
            if debug:
                nc.sync.dma_start(dbg["d_hsta"][:], HSTA[:])
                nc.sync.dma_start(dbg["d_hstb"][:], HSTB[:])
                nc.sync.dma_start(dbg["d_gr2"][:], GR2[:])
                nc.sync.dma_start(dbg["d_veb"][:], VEB[:])
                nc.sync.dma_start(dbg["d_rsa"][:], RSA[:])
                nc.sync.dma_start(dbg["d_w2"][:], W2G[:])
                nc.sync.dma_start(dbg["d_asb"][:], ASB[:])

        if repeats == 1:
            body()
        else:
            n_unrolled, rem = divmod(repeats, UNROLL)
            if n_unrolled:
                with tc.For_i(0, n_unrolled, 1):
                    for _ in range(UNROLL):
                        body()
            for _ in range(rem):
                body()

    nc.compile()
    return nc


def host_inputs(inputs, kc=None):
    """Slice/prepare per-core input maps from the full problem inputs.

    Host work is index bookkeeping only (occurrence slot lists, 0/1 group
    masks, group->sample collapse matrix) plus layout/dtype formatting; all
    tensor arithmetic on activation-sized data happens on device.

    Slots are one-per-masked-occurrence, packed contiguously sample after
    sample, so every slot weight is exactly its mask value and the gather
    weights reduce to host-known 0/1 masks.  A sample whose slot run
    straddles a 128-row tile boundary owns one group per touched tile;
    groups are collapsed to samples by a final 0/1 matmul.
    """
    hs12 = np.ascontiguousarray(np.asarray(inputs["hidden_states"])[12])  # [B,L,D]
    ts = np.asarray(inputs["token_starts"]).astype(np.int64)
    m = np.ascontiguousarray(np.asarray(inputs["aspect_in_text_mask"], dtype=np.float32))
    gw = np.ascontiguousarray(np.asarray(inputs["guid_W"], dtype=np.float32)[3])
    gb = np.asarray(inputs["guid_b"], dtype=np.float32)[3]
    ln_g = np.asarray(inputs["ln_g"], dtype=np.float32)
    ln_b = np.asarray(inputs["ln_b"], dtype=np.float32)
    cls_W = np.asarray(inputs["cls_W"], dtype=np.float32)
    cls_b = np.asarray(inputs["cls_b"], dtype=np.float32)

    # unique rows per sample with occurrence counts (index bookkeeping);
    # greedy-balance samples across cores so the max per-core slot total
    # (and hence the row-tile count) is minimal
    uniq, cnts = [], []
    for b in range(B):
        u, n = np.unique(ts[b][m[b] > 0], return_counts=True)
        uniq.append(u)
        cnts.append(n.astype(np.float32))
    sizes = np.array([len(u) for u in uniq])
    order = np.argsort(-sizes)
    loads = [0] * N_CORES
    counts = [0] * N_CORES
    perm = [[] for _ in range(N_CORES)]   # perm[c][s] = global sample index
    for b in order:
        c = min((c for c in range(N_CORES) if counts[c] < BC),
                key=lambda c: loads[c])
        perm[c].append(int(b))
        loads[c] += sizes[b]
        counts[c] += 1
    ntiles = max(3, -(-max(loads) // 128))
    nidx = ntiles * 128
    groups_per_tile = 6
    for c in range(N_CORES):
        bound = [0]
        for s in range(BC):
            bound.append(bound[-1] + sizes[perm[c][s]])
        for t in range(ntiles):
            ngt = sum(1 for s in range(BC)
                      if bound[s] < 128 * (t + 1) and bound[s + 1] > 128 * t)
            assert ngt <= groups_per_tile, f"core {c} tile {t}: {ngt} groups"

    clsw_eff = (ln_g[:, None] * cls_W).astype(np.float32)
    clsw_ext = np.concatenate([clsw_eff, -clsw_eff.sum(0)[None, :]], 0)  # mu row
    clsw_pad = np.zeros((640, 3), np.float32)
    for hc, (hlo, hhi) in enumerate(HCH):
        blk = clsw_ext[hlo:hhi].copy()
        if hc == 4:
            blk[:512 - 473] = 0.0   # overlap rows already counted in chunk 3
        clsw_pad[128 * hc:128 * hc + (hhi - hlo)] = blk
    clsb_eff = (ln_b @ cls_W + cls_b).astype(np.float32)
    clsb_rep = np.tile(clsb_eff[None, :], (BC, 1)).astype(np.float32)
    gwb = np.ascontiguousarray(
        gw.reshape(KT, 128, H).transpose(1, 0, 2)).astype(ml_dtypes.bfloat16)
    gbrow = gb[None, :].astype(ml_dtypes.bfloat16)
    onesrow = np.ones((1, 128), ml_dtypes.bfloat16)

    ng = groups_per_tile * ntiles
    idx_all = np.zeros((N_CORES, nidx), np.int16)
    smaskg_all = np.zeros((N_CORES, 128, ng), np.float32)
    coll_all = np.zeros((N_CORES, 128, BC), np.float32)
    for c in range(N_CORES):
        pos = 0
        spans = []
        wvals = np.zeros(nidx, np.float32)
        for s in range(BC):
            b = perm[c][s]
            lo, hi = pos, pos + sizes[b]
            idx_all[c, lo:hi] = (s * L + uniq[b]).astype(np.int16)
            wvals[lo:hi] = cnts[b]
            spans.append((lo, hi))
            pos = hi
        for t in range(ntiles):
            g = 0
            for s, (lo, hi) in enumerate(spans):
                tl, th = max(lo, 128 * t), min(hi, 128 * (t + 1))
                if tl >= th:
                    continue
                col = groups_per_tile * t + g
                smaskg_all[c, tl - 128 * t:th - 128 * t, col] =                     wvals[tl:th]
                coll_all[c, col, s] = 1.0
                g += 1
    # wrap gather list in 16 partitions: element k -> [k % 16, k // 16],
    # replicated into each Q7 core's 16-partition group
    wrap16 = idx_all.reshape(N_CORES, nidx // 16, 16).transpose(0, 2, 1)
    idx_wrap = np.ascontiguousarray(np.tile(wrap16, (1, 8, 1)))

    in_maps = []
    for c in range(N_CORES):
        sl = np.array(perm[c])
        in_maps.append(dict(
            hsb=np.ascontiguousarray(hs12[sl].reshape(BC * L, D)).astype(ml_dtypes.bfloat16),
            idx=idx_wrap[c],
            gwb=gwb,
            gbrow=gbrow,
            onesrow=onesrow,
            mnat=np.ascontiguousarray(m[sl]),
            smaskg=np.ascontiguousarray(smaskg_all[c]),
            coll=np.ascontiguousarray(coll_all[c]),
            clsw=clsw_pad,
            clsb=clsb_rep,
        ))
    return in_maps, ntiles, perm


_PROGRAMS = {}


def kernel(**inputs):
    in_maps, ntiles, perm = host_inputs(inputs)
    has_bias = bool(np.any(np.asarray(inputs["guid_b"], dtype=np.float32)[3]))
    key = (ntiles, has_bias)
    if key not in _PROGRAMS:
        _PROGRAMS[key] = build_program(repeats=1, ntiles_req=ntiles,
                                       has_bias=has_bias)
    nc = _PROGRAMS[key]
    res = run_bass_kernel_spmd(nc, in_maps, list(range(N_CORES)), trace=False)
    out = np.zeros((B, 3), np.float32)
    for c in range(N_CORES):
        out[np.array(perm[c])] = np.asarray(res.results[c]["out"], np.float32)
    return out



# revision 12
# speedup vs baseline: 1.1010x; 1.1010x over previous
"""Trainium2 Bass kernel for nn_BERT4GCN_53884659695997.

Mathematical reduction
----------------------
In the reference, ``feature`` is reassigned to ``LN(guidance)`` at the top of
every loop iteration, so the GCN block's output is never consumed; only the
last BERT layer's branch (index 3 -> hidden_states layer 12, which skips the
GCN block) reaches the output:

    t[b]      = LN(relu(hs[12,b][ts[b]] @ guid_W[3] + guid_b[3])) * ln_g + ln_b
    logits[b] = ((t[b] * m[b,:,None]).sum(0) / m[b].sum(0)) @ cls_W + cls_b

(verified numerically against the jax reference to ~7e-7 rel err).

Row gathers commute with the row-wise ops (matmul-by-row / relu / LN), so the
gather+mask folds into per-source-row weights.  One gather slot is allocated
per masked occurrence, packed contiguously sample after sample, so every
slot's reduction weight is host-known (mask count / mask sum); the host emits
only the slot list and per-(tile,sample) weight masks (pure index
bookkeeping; all tensor arithmetic on activation-sized data stays on device).

Device pipeline (per core, 8 samples), tuned for single-dispatch latency:
  1. ALL small constants (gather list, slot masks, folded classifier) ride in
     ONE byte-packed [128, ~224B] DMA issued first, so the gather index list
     is on-chip ~1us in; guid_W follows in two DMAs (HWDGE descriptor-gen is
     ~625ns per dma_start, serialized, so fewer DMAs = shorter critical path).
  2. A short chain of dummy matmuls on a memset tile warms the PE from t~0 so
     the DVFS p-state is at full clock when gathered data arrives (cold PE
     runs 3.7x slower for the first ~3us).
  3. Two half SWDGE ``dma_gather(transpose=True)`` ops (128 + 256 idx) pull
     only the needed rows (bf16) straight from HBM into the transposed
     [128, 6, n] layout the guidance matmul wants.
  4. bf16 guidance matmul per 128-row tile (full-rate PE), relu on the scalar
     engine in two column halves (overlaps LN stats), LN stats via DVE
     bn_stats/bn_aggr, rstd via scalar-engine Sqrt (same act table as Relu,
     eps folded into the activation bias) + one DVE reciprocal.
  5. Aspects accumulate PER SAMPLE directly in PSUM across row tiles
     (start/stop accumulation), so no slot-group bookkeeping, no collapse
     matmul.  LN affine + 1/sum(mask) fold host-side into the classifier
     weights / slot masks; the per-sample mean correction is an extra feature
     column (exact fp32 linear algebra).  Classifier bias lands via a K=1
     ones matmul so the tail is pure PE -> copy -> DMA.

Sharding: data-parallel over batch B=64 -> 8 samples per core on 8 cores.
"""

import numpy as np
import ml_dtypes
from contextlib import ExitStack

import concourse.bass as bass
import concourse.tile as tile
from concourse import bacc, mybir
from concourse.bass_utils import run_bass_kernel_spmd

F32 = mybir.dt.float32
BF16 = mybir.dt.bfloat16
I16 = mybir.dt.int16
U8 = mybir.dt.uint8
AX = mybir.AxisListType
ALU = mybir.AluOpType
ACTF = mybir.ActivationFunctionType

N_CORES = 8
B = 64
BC = B // N_CORES   # samples per core
L = 256
D = 768
H = 600
KT = D // 128       # 6 contraction tiles
EPS = 1e-5
HCH = ((0, 128), (128, 256), (256, 384), (384, 512), (473, 601))
# chunk 4 overlaps chunk 3 on h in [472, 512) so every aspect matmul is
# 128 partitions wide; the folded cls_W zeroes those rows in chunk 4
NCH = ((0, 512), (512, 600))   # PSUM-bank-aligned guidance column chunks
UNROLL = 16
N_WARM = 11         # PE p-state warmup matmuls


def _gather_splits(ntiles):
    """Gather op sizes in row tiles: first 128 idx alone (earliest first
    matmul), remainder in <=256-idx chunks (256-desc gathers pipeline well
    on HW; larger ones do not)."""
    splits = [1]
    rem = ntiles - 1
    while rem > 0:
        c = min(2, rem)
        splits.append(c)
        rem -= c
    return splits


def build_program(repeats: int = 1, ntiles_req: int = 3, has_bias: bool = False,
                  debug: bool = False):
    ntiles = ntiles_req
    nidx = ntiles * 128
    nc = bacc.Bacc("TRN2", target_bir_lowering=False, debug=False,
                   num_devices=N_CORES)

    # pack byte offsets (must match host_inputs)
    o_idx = 0
    o_sms = o_idx + 2 * (nidx // 16)
    o_clsw = o_sms + 4 * ntiles * BC
    o_clsb = o_clsw + 4 * 15
    pb = o_clsb + 12
    pb = (pb + 15) // 16 * 16
    o_gb = pb
    if has_bias:
        pb = o_gb + 2 * H + 2 * 128

    dr = {}
    def din(name, shape, dt=F32):
        dr[name] = nc.dram_tensor(name, list(shape), dt, kind="ExternalInput").ap()
    din("hsb", (BC * L, D), BF16)        # gather source, stays in HBM
    din("pack", (128, pb), U8)           # all small constants, one DMA
    din("gwb", (128, KT, H), BF16)       # guid_W[3] rearranged (k p) n -> p k n
    out_ap = nc.dram_tensor("out", [BC, 3], F32, kind="ExternalOutput").ap()
    dbg = {}
    if debug:
        for nm, shape, dt in [
            ("d_hst", (128, KT, 128 * ntiles_req), BF16),
            ("d_gr2", (128, ntiles_req, H + 2), BF16),
            ("d_mva", (128, ntiles_req, 2), F32),
            ("d_rsa", (128, ntiles_req), F32),
            ("d_w2", (128, ntiles_req, BC), BF16),
            ("d_asb", (128, 5, BC), F32)]:
            dbg[nm] = nc.dram_tensor(nm, list(shape), dt, kind="ExternalOutput").ap()

    splits = _gather_splits(ntiles)
    # tile t -> (gather buffer id, local tile offset)
    tmap = []
    for g, c in enumerate(splits):
        for j in range(c):
            tmap.append((g, j))

    with tile.TileContext(nc) as tc, ExitStack() as ctx:
        cpool = ctx.enter_context(tc.tile_pool(name="consts", bufs=1))
        hpool = ctx.enter_context(tc.tile_pool(name="hst", bufs=4))
        wpool = ctx.enter_context(tc.tile_pool(name="work", bufs=3))
        spool = ctx.enter_context(tc.tile_pool(name="small", bufs=4))
        pg_ps = ctx.enter_context(tc.tile_pool(name="pg", bufs=2, space="PSUM"))
        sm_ps = ctx.enter_context(tc.tile_pool(name="sm", bufs=2, space="PSUM"))
        wm_ps = ctx.enter_context(tc.tile_pool(name="wm", bufs=1, space="PSUM"))

        # ---- PE p-state warmup: dummy matmuls on a memset tile, issued
        # before anything else so the ramp overlaps the constant DMAs ----
        EPSC = cpool.tile([128, 1], F32, tag="epsc")
        nc.vector.memset(EPSC[:], EPS)
        WARM = cpool.tile([128, 512], BF16, tag="warm")
        nc.vector.memset(WARM[:], 0.0)
        # first activation is a Sqrt so the (single, hoisted) act-table load
        # picks the sqrt_and_others set, which also contains Relu and Copy
        SCR1 = cpool.tile([128, 1], F32, tag="scr1")
        nc.scalar.activation(SCR1[:], EPSC[:], ACTF.Sqrt)
        WARMP = wm_ps.tile([128, 512], F32, tag="warmp")
        for _ in range(N_WARM):
            nc.tensor.matmul(WARMP[:], WARM[:, 0:128], WARM[:],
                             start=True, stop=True)

        # ---- constants: pack first (holds the gather list), then weights ----
        PACK = cpool.tile([128, pb], U8, tag="pack")
        nc.sync.dma_start(PACK[:], dr["pack"][:])
        GWB = cpool.tile([128, KT, H], BF16, tag="gwb")
        nc.sync.dma_start(GWB[:, 0:3, :], dr["gwb"][:, 0:3, :])
        nc.sync.dma_start(GWB[:, 3:6, :], dr["gwb"][:, 3:6, :])
        ONES8 = cpool.tile([1, BC], F32, tag="ones8")
        nc.vector.memset(ONES8[:], 1.0)

        IDXT = PACK[:, o_idx:o_sms].bitcast(I16)                  # [128, nidx//16]
        SMS = PACK[:, o_sms:o_clsw].bitcast(F32).rearrange(
            "p (t s) -> p t s", s=BC)                             # [128, nt, BC]
        CLSW = PACK[:, o_clsw:o_clsb].bitcast(F32).rearrange(
            "p (c n) -> p c n", n=3)                              # [128, 5, 3]
        CLSB3 = PACK[0:1, o_clsb:o_clsb + 12].bitcast(F32)        # [1, 3]
        if has_bias:
            GBROW = PACK[0:1, o_gb:o_gb + 2 * H].bitcast(BF16)    # [1, H]
            ONESR = PACK[0:1, o_gb + 2 * H:o_gb + 2 * H + 256].bitcast(BF16)

        def body():
            # ---- gathered+transposed rows, straight from HBM ----
            HSTS = []
            off = 0
            for g, c in enumerate(splits):
                n = c * 128
                T = hpool.tile([128, KT, n], BF16, tag=f"hst{g}")
                nc.gpsimd.dma_gather(T[:], dr["hsb"][:],
                                     IDXT[:, off // 16:(off + n) // 16],
                                     n, n, D, transpose=True)
                HSTS.append(T)
                off += n

            GR2 = wpool.tile([128, ntiles, H + 2], BF16, tag="gr2")
            MVA = wpool.tile([128, ntiles, 2], F32, tag="mva")
            SD = wpool.tile([128, ntiles], F32, tag="sd")
            RSA = wpool.tile([128, ntiles], F32, tag="rsa")
            W2 = wpool.tile([128, ntiles, BC], BF16, tag="w2")
            # full-bank PSUM tile: matmul start=True zeroes (lazily) a whole
            # 2KB zero-region, so the accumulation bank must not be shared
            SMT = sm_ps.tile([128, 512], F32, tag="smt")
            ASPT = SMT[:, 0:5 * BC].rearrange("p (c s) -> p c s", s=BC)
            CL = SMT[0:BC, 5 * BC:5 * BC + 3]

            for t in range(ntiles):
                PG = pg_ps.tile([128, 1024], F32, tag="pg")
                g, tl = tmap[t]
                HST = HSTS[g]
                for nlo, nhi in NCH:
                    for k in range(KT):
                        nc.tensor.matmul(
                            PG[:, nlo:nhi], HST[:, k, 128 * tl:128 * (tl + 1)],
                            GWB[:, k, nlo:nhi], start=(k == 0),
                            stop=(not has_bias and k == KT - 1))
                    if has_bias:
                        nc.tensor.matmul(PG[:, nlo:nhi], ONESR[:],
                                         GBROW[:, nlo:nhi], start=False, stop=True)
                # relu in two halves so LN stats overlap the second half
                nc.scalar.activation(GR2[:, t, 0:300], PG[:, 0:300], ACTF.Relu)
                nc.scalar.activation(GR2[:, t, 300:600], PG[:, 300:600], ACTF.Relu)
                BST = spool.tile([128, 2, 6], BF16, tag="bst")
                nc.vector.bn_stats(BST[:, 0, :], GR2[:, t, 0:300])
                nc.vector.bn_stats(BST[:, 1, :], GR2[:, t, 300:600])
                nc.vector.bn_aggr(MVA[:, t, :], BST[:])
                # rstd = 1/sqrt(var+eps): Sqrt shares the Relu act table so
                # no extra table load; reciprocal runs accurately on DVE
                nc.scalar.activation(SD[:, t:t + 1], MVA[:, t, 1:2], ACTF.Sqrt,
                                     bias=EPSC[:])
                nc.scalar.activation(GR2[:, t, H:H + 1], MVA[:, t, 0:1],
                                     ACTF.Copy)     # mean feature column
                nc.vector.reciprocal(RSA[:, t:t + 1], SD[:, t:t + 1])
                nc.vector.tensor_tensor(
                    W2[:, t, :], SMS[:, t, :],
                    RSA[:, t:t + 1].broadcast_to((128, BC)), ALU.mult)
                # aspects accumulate per sample across row tiles in PSUM.
                # ONE start for the whole bank group: start=True lazily
                # zeroes the full 2KB zero-region, so a start per chunk
                # would wipe the other chunks' earlier contributions.
                for hc, (hlo, hhi) in enumerate(HCH):
                    nc.tensor.matmul(ASPT[0:hhi - hlo, hc, :],
                                     GR2[:, t, hlo:hhi], W2[:, t, :],
                                     start=(t == 0 and hc == 0),
                                     stop=(t == ntiles - 1 and hc == len(HCH) - 1),
                                     skip_group_check=True)

            # ---- classifier; bias via a K=1 ones matmul ----
            ASB = wpool.tile([128, 5, BC], F32, tag="asb")
            nc.scalar.activation(ASB[:], ASPT[:], ACTF.Copy)
            for hc, (hlo, hhi) in enumerate(HCH):
                nc.tensor.matmul(CL, ASB[0:hhi - hlo, hc, :],
                                 CLSW[0:hhi - hlo, hc, :],
                                 start=(hc == 0), stop=False)
            nc.tensor.matmul(CL, ONES8[:], CLSB3[:], start=False, stop=True)
            OSB = wpool.tile([BC, 3], F32, tag="osb")
            nc.scalar.activation(OSB[:], CL[:], ACTF.Copy)
            nc.sync.dma_start(out_ap[:], OSB[:])
            if debug:
                off = 0
                for g, cs in enumerate(splits):
                    n = cs * 128
                    nc.sync.dma_start(dbg["d_hst"][:, :, off:off + n], HSTS[g][:])
                    off += n
                nc.sync.dma_start(dbg["d_gr2"][:], GR2[:])
                nc.sync.dma_start(dbg["d_mva"][:], MVA[:])
                nc.sync.dma_start(dbg["d_rsa"][:], RSA[:])
                nc.sync.dma_start(dbg["d_w2"][:], W2[:])
                nc.sync.dma_start(dbg["d_asb"][:], ASB[:])

        if repeats == 1:
            body()
        else:
            n_unrolled, rem = divmod(repeats, UNROLL)
            if n_unrolled:
                with tc.For_i(0, n_unrolled, 1):
                    for _ in range(UNROLL):
                        body()
            for _ in range(rem):
                body()

    nc.compile()
    return nc


def host_inputs(inputs, kc=None):
    """Slice/prepare per-core input maps from the full problem inputs.

    Host work is index bookkeeping only (occurrence slot lists, per-sample
    weight masks) plus layout/dtype formatting; all tensor arithmetic on
    activation-sized data happens on device.
    """
    hs12 = np.ascontiguousarray(np.asarray(inputs["hidden_states"])[12])  # [B,L,D]
    ts = np.asarray(inputs["token_starts"]).astype(np.int64)
    m = np.ascontiguousarray(np.asarray(inputs["aspect_in_text_mask"], dtype=np.float32))
    gw = np.ascontiguousarray(np.asarray(inputs["guid_W"], dtype=np.float32)[3])
    gb = np.asarray(inputs["guid_b"], dtype=np.float32)[3]
    ln_g = np.asarray(inputs["ln_g"], dtype=np.float32)
    ln_b = np.asarray(inputs["ln_b"], dtype=np.float32)
    cls_W = np.asarray(inputs["cls_W"], dtype=np.float32)
    cls_b = np.asarray(inputs["cls_b"], dtype=np.float32)
    has_bias = bool(np.any(gb))

    # unique rows per sample with occurrence counts (index bookkeeping);
    # greedy-balance samples across cores so the max per-core slot total
    # (and hence the row-tile count) is minimal
    uniq, cnts = [], []
    for b in range(B):
        u, n = np.unique(ts[b][m[b] > 0], return_counts=True)
        uniq.append(u)
        cnts.append(n.astype(np.float32))
    msum = m.sum(1)  # per-sample mask sum, folded into the slot weights
    sizes = np.array([len(u) for u in uniq])
    order = np.argsort(-sizes)
    loads = [0] * N_CORES
    counts = [0] * N_CORES
    perm = [[] for _ in range(N_CORES)]   # perm[c][s] = global sample index
    for b in order:
        c = min((c for c in range(N_CORES) if counts[c] < BC),
                key=lambda c: loads[c])
        perm[c].append(int(b))
        loads[c] += sizes[b]
        counts[c] += 1
    ntiles = max(3, -(-max(loads) // 128))
    nidx = ntiles * 128

    # folded classifier: LN affine into cls_W, mean correction as an extra
    # feature row (row 600), packed into 5 partition chunks with the overlap
    # rows of chunk 4 zeroed
    clsw_eff = (ln_g[:, None] * cls_W).astype(np.float32)
    clsw_ext = np.concatenate([clsw_eff, -clsw_eff.sum(0)[None, :]], 0)
    clsw_pack = np.zeros((128, 5, 3), np.float32)
    for hc, (hlo, hhi) in enumerate(HCH):
        blk = clsw_ext[hlo:hhi].copy()
        if hc == 4:
            blk[:512 - 473] = 0.0   # overlap rows already counted in chunk 3
        clsw_pack[0:hhi - hlo, hc] = blk
    clsb_eff = (ln_b @ cls_W + cls_b).astype(np.float32)
    gwb = np.ascontiguousarray(
        gw.reshape(KT, 128, H).transpose(1, 0, 2)).astype(ml_dtypes.bfloat16)

    # pack byte offsets (must match build_program)
    o_idx = 0
    o_sms = o_idx + 2 * (nidx // 16)
    o_clsw = o_sms + 4 * ntiles * BC
    o_clsb = o_clsw + 4 * 15
    pb = o_clsb + 12
    pb = (pb + 15) // 16 * 16
    o_gb = pb
    if has_bias:
        pb = o_gb + 2 * H + 2 * 128

    in_maps = []
    for c in range(N_CORES):
        idx = np.zeros(nidx, np.int16)
        sms = np.zeros((128, ntiles, BC), np.float32)
        pos = 0
        for s in range(BC):
            b = perm[c][s]
            lo, hi = pos, pos + sizes[b]
            idx[lo:hi] = (s * L + uniq[b]).astype(np.int16)
            w = cnts[b] / msum[b]
            for t in range(lo // 128, (hi + 127) // 128):
                tl, th = max(lo, 128 * t), min(hi, 128 * (t + 1))
                sms[tl - 128 * t:th - 128 * t, t, s] = w[tl - lo:th - lo]
            pos = hi
        # wrap gather list in 16 partitions: element k -> [k % 16, k // 16],
        # replicated into each Q7 core's 16-partition group
        wrap16 = idx.reshape(nidx // 16, 16).T
        idx_wrap = np.ascontiguousarray(np.tile(wrap16, (8, 1)))  # [128, nidx//16]

        pack = np.zeros((128, pb), np.uint8)
        pack[:, o_idx:o_sms] = idx_wrap.view(np.uint8).reshape(128, -1)
        pack[:, o_sms:o_clsw] = sms.reshape(128, -1).view(np.uint8)
        pack[:, o_clsw:o_clsb] = clsw_pack.reshape(128, -1).view(np.uint8)
        pack[0, o_clsb:o_clsb + 12] = clsb_eff.view(np.uint8)
        if has_bias:
            pack[0, o_gb:o_gb + 2 * H] = gb.astype(ml_dtypes.bfloat16).view(np.uint8)
            pack[0, o_gb + 2 * H:o_gb + 2 * H + 256] = \
                np.ones(128, ml_dtypes.bfloat16).view(np.uint8)

        sl = np.array(perm[c])
        in_maps.append(dict(
            hsb=np.ascontiguousarray(hs12[sl].reshape(BC * L, D)).astype(ml_dtypes.bfloat16),
            pack=pack,
            gwb=gwb,
        ))
    return in_maps, ntiles, perm


_PROGRAMS = {}


def kernel(**inputs):
    in_maps, ntiles, perm = host_inputs(inputs)
    has_bias = bool(np.any(np.asarray(inputs["guid_b"], dtype=np.float32)[3]))
    key = (ntiles, has_bias)
    if key not in _PROGRAMS:
        _PROGRAMS[key] = build_program(repeats=1, ntiles_req=ntiles,
                                       has_bias=has_bias)
    nc = _PROGRAMS[key]
    res = run_bass_kernel_spmd(nc, in_maps, list(range(N_CORES)), trace=False)
    out = np.zeros((B, 3), np.float32)
    for c in range(N_CORES):
        out[np.array(perm[c])] = np.asarray(res.results[c]["out"], np.float32)
    return out


# revision 14
# speedup vs baseline: 1.5274x; 1.3872x over previous
"""Trainium2 Bass kernel for nn_BERT4GCN_53884659695997.

Mathematical reduction
----------------------
In the reference, ``feature`` is reassigned to ``LN(guidance)`` at the top of
every loop iteration, so the GCN block's output is never consumed; only the
last BERT layer's branch (index 3 -> hidden_states layer 12, which skips the
GCN block) reaches the output:

    t[b]      = LN(relu(hs[12,b][ts[b]] @ guid_W[3] + guid_b[3])) * ln_g + ln_b
    logits[b] = ((t[b] * m[b,:,None]).sum(0) / m[b].sum(0)) @ cls_W + cls_b

(verified numerically against the jax reference to ~7e-7 rel err).

Row gathers commute with the row-wise ops (matmul-by-row / relu / LN), so the
gather+mask folds into per-source-row weights.  One gather slot is allocated
per masked occurrence, packed contiguously sample after sample, so every
slot's reduction weight is host-known (mask count / mask sum); the host emits
only the slot list and per-(tile,sample) weight masks (pure index
bookkeeping; all tensor arithmetic on activation-sized data stays on device).

Device pipeline (per core, 8 samples), tuned for single-dispatch latency:
  1. ALL small constants (gather list, slot masks, folded classifier) ride in
     ONE byte-packed [128, ~224B] DMA issued first, so the gather index list
     is on-chip ~1us in; guid_W follows in two DMAs (HWDGE descriptor-gen is
     ~625ns per dma_start, serialized, so fewer DMAs = shorter critical path).
  2. A short chain of dummy matmuls on a memset tile warms the PE from t~0 so
     the DVFS p-state is at full clock when gathered data arrives (cold PE
     runs 3.7x slower for the first ~3us).
  3. Two half SWDGE ``dma_gather(transpose=True)`` ops (128 + 256 idx) pull
     only the needed rows (bf16) straight from HBM into the transposed
     [128, 6, n] layout the guidance matmul wants.
  4. bf16 guidance matmul per 128-row tile (full-rate PE), relu on the scalar
     engine in two column halves (overlaps LN stats), LN stats via DVE
     bn_stats/bn_aggr, rstd via scalar-engine Sqrt (same act table as Relu,
     eps folded into the activation bias) + one DVE reciprocal.
  5. Aspects accumulate PER SAMPLE directly in PSUM across row tiles
     (start/stop accumulation), so no slot-group bookkeeping, no collapse
     matmul.  LN affine + 1/sum(mask) fold host-side into the classifier
     weights / slot masks; the per-sample mean correction is an extra feature
     column (exact fp32 linear algebra).  Classifier bias lands via a K=1
     ones matmul so the tail is pure PE -> copy -> DMA.

Sharding: data-parallel over batch B=64 -> 8 samples per core on 8 cores.
"""

import numpy as np
import ml_dtypes
from contextlib import ExitStack

import concourse.bass as bass
import concourse.tile as tile
from concourse import bacc, mybir
from concourse.bass_utils import run_bass_kernel_spmd

F32 = mybir.dt.float32
BF16 = mybir.dt.bfloat16
I16 = mybir.dt.int16
U8 = mybir.dt.uint8
AX = mybir.AxisListType
ALU = mybir.AluOpType
ACTF = mybir.ActivationFunctionType

N_CORES = 8
B = 64
BC = B // N_CORES   # samples per core
L = 256
D = 768
H = 600
KT = D // 128       # 6 contraction tiles
EPS = 1e-5
HCH = ((0, 128), (128, 256), (256, 384), (384, 512), (473, 601))
# chunk 4 overlaps chunk 3 on h in [472, 512) so every aspect matmul is
# 128 partitions wide; the folded cls_W zeroes those rows in chunk 4
NCH = ((0, 512), (512, 600))   # PSUM-bank-aligned guidance column chunks
UNROLL = 16
N_WARM = 11         # PE p-state warmup matmuls


def _gather_splits(ntiles):
    """Gather op sizes in row tiles: first 128 idx alone (earliest first
    matmul), remainder in <=256-idx chunks (256-desc gathers pipeline well
    on HW; larger ones do not)."""
    splits = [1]
    rem = ntiles - 1
    while rem > 0:
        c = min(2, rem)
        splits.append(c)
        rem -= c
    return splits


def build_program(repeats: int = 1, ntiles_req: int = 3, has_bias: bool = False,
                  debug: bool = False):
    ntiles = ntiles_req
    nidx = ntiles * 128
    nc = bacc.Bacc("TRN2", target_bir_lowering=False, debug=False,
                   num_devices=N_CORES)

    # pack byte offsets (must match host_inputs)
    o_idx = 0
    o_sms = o_idx + 2 * (nidx // 16)
    o_clsw = o_sms + 4 * ntiles * BC
    o_clsb = o_clsw + 4 * 15
    pb = o_clsb + 12
    pb = (pb + 15) // 16 * 16
    o_gb = pb
    if has_bias:
        pb = o_gb + 2 * H + 2 * 128

    dr = {}
    def din(name, shape, dt=F32):
        dr[name] = nc.dram_tensor(name, list(shape), dt, kind="ExternalInput").ap()
    din("hsb", (BC * L, D), BF16)        # gather source, stays in HBM
    din("pack", (128, pb), U8)           # all small constants, one DMA
    din("gwb", (128, KT, H), BF16)       # guid_W[3] rearranged (k p) n -> p k n
    out_ap = nc.dram_tensor("out", [BC, 3], F32, kind="ExternalOutput").ap()
    dbg = {}
    if debug:
        for nm, shape, dt in [
            ("d_hst", (128, KT, 128 * ntiles_req), BF16),
            ("d_gr2", (128, ntiles_req, H + 2), BF16),
            ("d_mva", (128, ntiles_req, 2), F32),
            ("d_rsa", (128, ntiles_req), F32),
            ("d_w2", (128, ntiles_req, BC), BF16),
            ("d_asb", (128, 5, BC), F32)]:
            dbg[nm] = nc.dram_tensor(nm, list(shape), dt, kind="ExternalOutput").ap()

    splits = _gather_splits(ntiles)
    # tile t -> (gather buffer id, local tile offset)
    tmap = []
    for g, c in enumerate(splits):
        for j in range(c):
            tmap.append((g, j))

    with tile.TileContext(nc) as tc, ExitStack() as ctx:
        cpool = ctx.enter_context(tc.tile_pool(name="consts", bufs=1))
        hpool = ctx.enter_context(tc.tile_pool(name="hst", bufs=4))
        wpool = ctx.enter_context(tc.tile_pool(name="work", bufs=3))
        spool = ctx.enter_context(tc.tile_pool(name="small", bufs=4))
        pg_ps = ctx.enter_context(tc.tile_pool(name="pg", bufs=2, space="PSUM"))
        sm_ps = ctx.enter_context(tc.tile_pool(name="sm", bufs=2, space="PSUM"))
        wm_ps = ctx.enter_context(tc.tile_pool(name="wm", bufs=1, space="PSUM"))

        # ---- PE p-state warmup: dummy matmuls on a memset tile, issued
        # before anything else so the ramp overlaps the constant DMAs ----
        EPSC = cpool.tile([128, 1], F32, tag="epsc")
        nc.vector.memset(EPSC[:], EPS)
        WARM = cpool.tile([128, 512], BF16, tag="warm")
        nc.vector.memset(WARM[:], 0.0)
        # first activation is a Sqrt so the (single, hoisted) act-table load
        # picks the sqrt_and_others set, which also contains Relu and Copy
        SCR1 = cpool.tile([128, 1], F32, tag="scr1")
        nc.scalar.activation(SCR1[:], EPSC[:], ACTF.Sqrt)
        WARMP = wm_ps.tile([128, 512], F32, tag="warmp")
        for _ in range(N_WARM):
            nc.tensor.matmul(WARMP[:], WARM[:, 0:128], WARM[:],
                             start=True, stop=True)

        # ---- constants: pack first (holds the gather list), then weights ----
        PACK = cpool.tile([128, pb], U8, tag="pack")
        nc.sync.dma_start(PACK[:], dr["pack"][:])
        GWB = cpool.tile([128, KT, H], BF16, tag="gwb")
        nc.sync.dma_start(GWB[:, 0:3, :], dr["gwb"][:, 0:3, :])
        nc.sync.dma_start(GWB[:, 3:6, :], dr["gwb"][:, 3:6, :])
        ONES8 = cpool.tile([1, BC], F32, tag="ones8")
        nc.vector.memset(ONES8[:], 1.0)

        IDXT = PACK[:, o_idx:o_sms].bitcast(I16)                  # [128, nidx//16]
        SMS = PACK[:, o_sms:o_clsw].bitcast(F32).rearrange(
            "p (t s) -> p t s", s=BC)                             # [128, nt, BC]
        CLSW = PACK[:, o_clsw:o_clsb].bitcast(F32).rearrange(
            "p (c n) -> p c n", n=3)                              # [128, 5, 3]
        CLSB3 = PACK[0:1, o_clsb:o_clsb + 12].bitcast(F32)        # [1, 3]
        if has_bias:
            GBROW = PACK[0:1, o_gb:o_gb + 2 * H].bitcast(BF16)    # [1, H]
            ONESR = PACK[0:1, o_gb + 2 * H:o_gb + 2 * H + 256].bitcast(BF16)

        def body(pending):
            """Emit one sample-batch iteration.

            The aspect matmuls run one guidance tile behind (their W2 input
            is ready ~2us after the tile's guidance matmul), and the
            last-tile aspects + classifier of the PREVIOUS body are
            interleaved between this body's guidance tiles, so the in-order
            PE never stalls waiting on the relu/LN-stats chain.  Returns the
            deferred stages for the next body (or flush) to emit.
            """
            HSTS = []
            off = 0
            for g, c in enumerate(splits):
                n = c * 128
                T = hpool.tile([128, KT, n], BF16, tag=f"hst{g}")
                nc.gpsimd.dma_gather(T[:], dr["hsb"][:],
                                     IDXT[:, off // 16:(off + n) // 16],
                                     n, n, D, transpose=True)
                HSTS.append(T)
                off += n

            GR2 = wpool.tile([128, ntiles, H + 2], BF16, tag="gr2")
            MVA = wpool.tile([128, ntiles, 2], F32, tag="mva")
            SD = wpool.tile([128, ntiles], F32, tag="sd")
            RSA = wpool.tile([128, ntiles], F32, tag="rsa")
            W2 = wpool.tile([128, ntiles, BC], BF16, tag="w2")
            # full-bank PSUM tile: matmul start=True zeroes (lazily) a whole
            # 2KB zero-region, so the accumulation bank must not be shared
            SMT = sm_ps.tile([128, 512], F32, tag="smt")
            ASPT = SMT[:, 0:5 * BC].rearrange("p (c s) -> p c s", s=BC)
            CL = SMT[0:BC, 5 * BC:5 * BC + 3]

            def guidance(t):
                PG = pg_ps.tile([128, 1024], F32, tag="pg")
                g, tl = tmap[t]
                HST = HSTS[g]
                for nlo, nhi in NCH:
                    for k in range(KT):
                        nc.tensor.matmul(
                            PG[:, nlo:nhi], HST[:, k, 128 * tl:128 * (tl + 1)],
                            GWB[:, k, nlo:nhi], start=(k == 0),
                            stop=(not has_bias and k == KT - 1))
                    if has_bias:
                        nc.tensor.matmul(PG[:, nlo:nhi], ONESR[:],
                                         GBROW[:, nlo:nhi], start=False, stop=True)
                # relu halves run on Act and DVE in parallel; the DVE half's
                # bn_stats follows in-order on the same engine (no handoff)
                nc.scalar.activation(GR2[:, t, 0:300], PG[:, 0:300], ACTF.Relu)
                nc.vector.tensor_scalar(GR2[:, t, 300:600], PG[:, 300:600],
                                        0.0, None, ALU.max)
                BST = spool.tile([128, 2, 6], BF16, tag="bst")
                nc.vector.bn_stats(BST[:, 1, :], GR2[:, t, 300:600])
                nc.vector.bn_stats(BST[:, 0, :], GR2[:, t, 0:300])
                nc.vector.bn_aggr(MVA[:, t, :], BST[:])
                # rstd = 1/sqrt(var+eps): Sqrt shares the Relu act table so
                # no extra table load; reciprocal runs accurately on DVE
                nc.scalar.activation(SD[:, t:t + 1], MVA[:, t, 1:2], ACTF.Sqrt,
                                     bias=EPSC[:])
                nc.scalar.activation(GR2[:, t, H:H + 1], MVA[:, t, 0:1],
                                     ACTF.Copy)     # mean feature column
                nc.vector.reciprocal(RSA[:, t:t + 1], SD[:, t:t + 1])
                nc.vector.tensor_tensor(
                    W2[:, t, :], SMS[:, t, :],
                    RSA[:, t:t + 1].broadcast_to((128, BC)), ALU.mult)

            def aspects(t):
                # aspects accumulate per sample across row tiles in PSUM.
                # ONE start for the whole bank group: start=True lazily
                # zeroes the full 2KB zero-region, so a start per chunk
                # would wipe the other chunks' earlier contributions.
                for hc, (hlo, hhi) in enumerate(HCH):
                    nc.tensor.matmul(ASPT[0:hhi - hlo, hc, :],
                                     GR2[:, t, hlo:hhi], W2[:, t, :],
                                     start=(t == 0 and hc == 0),
                                     stop=(t == ntiles - 1 and hc == len(HCH) - 1),
                                     skip_group_check=True)

            ASB = wpool.tile([128, 5, BC], F32, tag="asb")

            def stage_a():
                aspects(ntiles - 1)
                nc.scalar.activation(ASB[:], ASPT[:], ACTF.Copy)

            def stage_b():
                # classifier; bias via a K=1 ones matmul
                for hc, (hlo, hhi) in enumerate(HCH):
                    nc.tensor.matmul(CL, ASB[0:hhi - hlo, hc, :],
                                     CLSW[0:hhi - hlo, hc, :],
                                     start=(hc == 0), stop=False)
                nc.tensor.matmul(CL, ONES8[:], CLSB3[:], start=False, stop=True)
                OSB = wpool.tile([BC, 3], F32, tag="osb")
                nc.scalar.activation(OSB[:], CL[:], ACTF.Copy)
                nc.sync.dma_start(out_ap[:], OSB[:])
                if debug:
                    off = 0
                    for g, cs in enumerate(splits):
                        n = cs * 128
                        nc.sync.dma_start(dbg["d_hst"][:, :, off:off + n],
                                          HSTS[g][:])
                        off += n
                    nc.sync.dma_start(dbg["d_gr2"][:], GR2[:])
                    nc.sync.dma_start(dbg["d_mva"][:], MVA[:])
                    nc.sync.dma_start(dbg["d_rsa"][:], RSA[:])
                    nc.sync.dma_start(dbg["d_w2"][:], W2[:])
                    nc.sync.dma_start(dbg["d_asb"][:], ASB[:])

            guidance(0)
            if pending:
                pending[0]()
            guidance(1)
            if pending:
                pending[1]()
            aspects(0)
            for t in range(2, ntiles):
                guidance(t)
                aspects(t - 1)
            return (stage_a, stage_b)

        def run_chain(n):
            pending = None
            for _ in range(n):
                pending = body(pending)
            pending[0]()
            pending[1]()

        if repeats == 1:
            run_chain(1)
        else:
            n_unrolled, rem = divmod(repeats, UNROLL)
            if n_unrolled:
                with tc.For_i(0, n_unrolled, 1):
                    run_chain(UNROLL)
            if rem:
                run_chain(rem)

    nc.compile()
    return nc


def host_inputs(inputs, kc=None):
    """Slice/prepare per-core input maps from the full problem inputs.

    Host work is index bookkeeping only (occurrence slot lists, per-sample
    weight masks) plus layout/dtype formatting; all tensor arithmetic on
    activation-sized data happens on device.
    """
    hs12 = np.ascontiguousarray(np.asarray(inputs["hidden_states"])[12])  # [B,L,D]
    ts = np.asarray(inputs["token_starts"]).astype(np.int64)
    m = np.ascontiguousarray(np.asarray(inputs["aspect_in_text_mask"], dtype=np.float32))
    gw = np.ascontiguousarray(np.asarray(inputs["guid_W"], dtype=np.float32)[3])
    gb = np.asarray(inputs["guid_b"], dtype=np.float32)[3]
    ln_g = np.asarray(inputs["ln_g"], dtype=np.float32)
    ln_b = np.asarray(inputs["ln_b"], dtype=np.float32)
    cls_W = np.asarray(inputs["cls_W"], dtype=np.float32)
    cls_b = np.asarray(inputs["cls_b"], dtype=np.float32)
    has_bias = bool(np.any(gb))

    # unique rows per sample with occurrence counts (index bookkeeping);
    # greedy-balance samples across cores so the max per-core slot total
    # (and hence the row-tile count) is minimal
    uniq, cnts = [], []
    for b in range(B):
        u, n = np.unique(ts[b][m[b] > 0], return_counts=True)
        uniq.append(u)
        cnts.append(n.astype(np.float32))
    msum = m.sum(1)  # per-sample mask sum, folded into the slot weights
    sizes = np.array([len(u) for u in uniq])
    order = np.argsort(-sizes)
    loads = [0] * N_CORES
    counts = [0] * N_CORES
    perm = [[] for _ in range(N_CORES)]   # perm[c][s] = global sample index
    for b in order:
        c = min((c for c in range(N_CORES) if counts[c] < BC),
                key=lambda c: loads[c])
        perm[c].append(int(b))
        loads[c] += sizes[b]
        counts[c] += 1
    ntiles = max(3, -(-max(loads) // 128))
    nidx = ntiles * 128

    # folded classifier: LN affine into cls_W, mean correction as an extra
    # feature row (row 600), packed into 5 partition chunks with the overlap
    # rows of chunk 4 zeroed
    clsw_eff = (ln_g[:, None] * cls_W).astype(np.float32)
    clsw_ext = np.concatenate([clsw_eff, -clsw_eff.sum(0)[None, :]], 0)
    clsw_pack = np.zeros((128, 5, 3), np.float32)
    for hc, (hlo, hhi) in enumerate(HCH):
        blk = clsw_ext[hlo:hhi].copy()
        if hc == 4:
            blk[:512 - 473] = 0.0   # overlap rows already counted in chunk 3
        clsw_pack[0:hhi - hlo, hc] = blk
    clsb_eff = (ln_b @ cls_W + cls_b).astype(np.float32)
    gwb = np.ascontiguousarray(
        gw.reshape(KT, 128, H).transpose(1, 0, 2)).astype(ml_dtypes.bfloat16)

    # pack byte offsets (must match build_program)
    o_idx = 0
    o_sms = o_idx + 2 * (nidx // 16)
    o_clsw = o_sms + 4 * ntiles * BC
    o_clsb = o_clsw + 4 * 15
    pb = o_clsb + 12
    pb = (pb + 15) // 16 * 16
    o_gb = pb
    if has_bias:
        pb = o_gb + 2 * H + 2 * 128

    in_maps = []
    for c in range(N_CORES):
        idx = np.zeros(nidx, np.int16)
        sms = np.zeros((128, ntiles, BC), np.float32)
        pos = 0
        for s in range(BC):
            b = perm[c][s]
            lo, hi = pos, pos + sizes[b]
            idx[lo:hi] = (s * L + uniq[b]).astype(np.int16)
            w = cnts[b] / msum[b]
            for t in range(lo // 128, (hi + 127) // 128):
                tl, th = max(lo, 128 * t), min(hi, 128 * (t + 1))
                sms[tl - 128 * t:th - 128 * t, t, s] = w[tl - lo:th - lo]
            pos = hi
        # wrap gather list in 16 partitions: element k -> [k % 16, k // 16],
        # replicated into each Q7 core's 16-partition group
        wrap16 = idx.reshape(nidx // 16, 16).T
        idx_wrap = np.ascontiguousarray(np.tile(wrap16, (8, 1)))  # [128, nidx//16]

        pack = np.zeros((128, pb), np.uint8)
        pack[:, o_idx:o_sms] = idx_wrap.view(np.uint8).reshape(128, -1)
        pack[:, o_sms:o_clsw] = sms.reshape(128, -1).view(np.uint8)
        pack[:, o_clsw:o_clsb] = clsw_pack.reshape(128, -1).view(np.uint8)
        pack[0, o_clsb:o_clsb + 12] = clsb_eff.view(np.uint8)
        if has_bias:
            pack[0, o_gb:o_gb + 2 * H] = gb.astype(ml_dtypes.bfloat16).view(np.uint8)
            pack[0, o_gb + 2 * H:o_gb + 2 * H + 256] = \
                np.ones(128, ml_dtypes.bfloat16).view(np.uint8)

        sl = np.array(perm[c])
        in_maps.append(dict(
            hsb=np.ascontiguousarray(hs12[sl].reshape(BC * L, D)).astype(ml_dtypes.bfloat16),
            pack=pack,
            gwb=gwb,
        ))
    return in_maps, ntiles, perm


_PROGRAMS = {}


def kernel(**inputs):
    in_maps, ntiles, perm = host_inputs(inputs)
    has_bias = bool(np.any(np.asarray(inputs["guid_b"], dtype=np.float32)[3]))
    key = (ntiles, has_bias)
    if key not in _PROGRAMS:
        _PROGRAMS[key] = build_program(repeats=1, ntiles_req=ntiles,
                                       has_bias=has_bias)
    nc = _PROGRAMS[key]
    res = run_bass_kernel_spmd(nc, in_maps, list(range(N_CORES)), trace=False)
    out = np.zeros((B, 3), np.float32)
    for c in range(N_CORES):
        out[np.array(perm[c])] = np.asarray(res.results[c]["out"], np.float32)
    return out
